# revision 60
# baseline (speedup 1.0000x reference)
"""Trainium2 Bass kernel for nn_DenseInputEncoder (to_dense_adj-style scatter).

Strategy (data-parallel over graphs, 8 graphs per NeuronCore):
  The output dense_pair_h[b, h, r, c] is a mostly-zero dense tensor built from
  ~2k scattered (r, c) cells per graph.  Instead of DMA scatter, each cell
  sub-range of the output is produced DENSE by a TensorE matmul:
      out[h, cell] = sum_items V[item, h] * onehot[item, cell]
  where onehot[item, cell] = (rc_local[item] == iota[cell]) is built by
  tensor_scalar is_equal ops (int16 iota vs f32 per-partition scalar -> fp16,
  DVE 4x mode; a quarter go to the otherwise idle GpSimd).  The matmul
  simultaneously performs the scatter, sums duplicate cells, and fills
  untouched cells with exact zeros.  Item values V = feat96 @ W96 unify the
  edge/pair/node-diagonal encoders (features are placed in disjoint 96-dim
  blocks on the host, so one weight matrix serves all three).

  Work layout: cells are compacted to r*64+c when all c < 64 (true for this
  data), 512-cell PSUM windows are subdivided into ~171-cell sub-ranges so a
  (graph, window, sub) group usually fits one 128-slot chunk, and host-side
  planning pads groups uniformly across cores so one SPMD program serves all
  8 cores.  Graph pairs share [128 x live] fp16 SBUF slabs (partitions 0-63 =
  graph a's h-planes, 64-127 = graph b's) DMA'd out in ~1 MiB blocks as
  windows complete; input DMAs ride the separate ACT HWDGE queue.  Only the
  live [r < rl, c < 64] block is computed on device (fp16); the host converts
  to f32 and pads the structurally-zero remainder.
"""

import numpy as np
import ml_dtypes
from contextlib import ExitStack

import concourse.mybir as mybir
import concourse.tile as tile
from concourse import bacc
from concourse.bass_utils import run_bass_kernel_spmd

B = 64          # graphs
N = 128         # max nodes per graph (dense padding)
H = 64          # hidden dim
NCORES = 8
GPC = B // NCORES  # graphs per core
WIN = 512       # cells per window (one PSUM bank at fp32)
P = 128         # partitions / matmul contraction size
F = 96          # unified feature dim: [edge 32 | pair 16 | node 32 | loop 16]

_f32 = mybir.dt.float32
_bf16 = mybir.dt.bfloat16
_i16 = mybir.dt.int16
_f16 = mybir.dt.float16

_program_cache = {}


def _host_prep(inputs):
    """Index math + feature packing on host (numpy).  Returns per-core input
    arrays, the uniform chunk plan, and the (host-computed) node mask."""
    batch = np.asarray(inputs["batch"]).astype(np.int64)
    edge_index = np.asarray(inputs["edge_index"]).astype(np.int64)
    pair_index = np.asarray(inputs["pair_index"]).astype(np.int64)
    node_x = np.asarray(inputs["node_x"], dtype=np.float32)
    loop_x = np.asarray(inputs["loop_x"], dtype=np.float32)
    edge_attr = np.asarray(inputs["edge_attr"], dtype=np.float32)
    pair_x = np.asarray(inputs["pair_x"], dtype=np.float32)

    NT = batch.shape[0]
    E = edge_index.shape[1]

    # position of each node within its graph (to_dense_batch semantics)
    counts = np.bincount(batch, minlength=B)
    starts = np.concatenate([[0], np.cumsum(counts)[:-1]])
    pos = np.arange(NT, dtype=np.int64) - starts[batch]

    # unified item list: edges, pairs, node-diagonal entries
    e0, e1 = edge_index
    p0, p1 = pair_index
    b_it = np.concatenate([batch[e0], batch[p0], batch])
    r_it = np.concatenate([pos[e0], pos[p0], pos])
    c_it = np.concatenate([pos[e1], pos[p1], pos])
    n_items = b_it.shape[0]

    feat = np.zeros((n_items, F), np.float32)
    feat[:E, 0:32] = edge_attr
    feat[E : 2 * E, 32:48] = pair_x
    feat[2 * E :, 48:80] = node_x
    feat[2 * E :, 80:96] = loop_x

    # out-of-bounds scatter indices are dropped (jax .at[] default)
    valid = (r_it >= 0) & (r_it < N) & (c_it >= 0) & (c_it < N) & (b_it >= 0) & (b_it < B)
    b_v, r_v, c_v = b_it[valid], r_it[valid], c_it[valid]
    feat_v = feat[valid]

    # column compaction: if all c < 64, use a r*64+c cell space (halves the
    # number of scatter windows); the slab copy re-expands.
    cw = 64 if (c_v.size == 0 or c_v.max() < 64) else N
    nwin = (N * cw) // WIN
    cell = r_v * cw + c_v
    w_v = cell // WIN
    rc_local = (cell % WIN).astype(np.float32)
    core_v = b_v // GPC
    g_v = b_v % GPC

    # Subdivide each 512-cell window into SUB cell sub-ranges so that one
    # (g, w, sub) group usually fits a single 128-slot chunk — narrower
    # sub-ranges mean narrower scatter matmuls and one-hot compares.  Pick
    # SUB by a PE-cost proxy over the candidates.
    in_w = (cell % WIN).astype(np.int64)
    best = None
    for sub_try in (1, 2, 3, 4):
        bounds = [round(s * WIN / sub_try) for s in range(sub_try + 1)]
        widths = np.diff(bounds)
        sub_v_t = np.minimum(
            np.searchsorted(bounds, in_w, side="right") - 1, sub_try - 1
        )
        key_t = ((core_v * GPC + g_v) * nwin + w_v) * sub_try + sub_v_t
        cnt_t = np.bincount(
            key_t, minlength=NCORES * GPC * nwin * sub_try
        ).reshape(NCORES, GPC, nwin, sub_try)
        C_t = -(-cnt_t.max(axis=0) // P)  # [GPC, nwin, sub]
        pe_cost = int((C_t * (P + widths[None, None, :])).sum())
        if best is None or pe_cost < best[0]:
            best = (pe_cost, sub_try, bounds, sub_v_t, key_t, C_t)
    _, SUB, bounds, sub_v, key, C_gws = best

    # chunk table: chunk ids ordered by consumption — (pair, w, graph, sub,
    # cc) — so the one-hot builds and matmuls stream in the same order.
    # Chunks carry exact slot counts (K <= 128): feats slots are packed, not
    # padded to 128 per chunk.
    S_gws = np.bincount(key, minlength=NCORES * GPC * nwin * SUB).reshape(
        NCORES, GPC, nwin, SUB
    ).max(axis=0)
    t_start = np.zeros((GPC, nwin, SUB), np.int64)
    slot_start = np.zeros((GPC, nwin, SUB), np.int64)
    plan = []  # per pair: (t0, soff0, [(w, [(gg, lo, hi, [(t, soff, K)...])])])
    T = 0
    S_total = 0
    for pair in range(GPC // 2):
        t0_pair = T
        soff_pair = S_total
        wplans = []
        for w in range(nwin):
            went = []
            for gg in range(2):
                g = 2 * pair + gg
                for sub in range(SUB):
                    s_ = int(S_gws[g, w, sub])
                    if s_ == 0:
                        continue
                    t_start[g, w, sub] = T
                    slot_start[g, w, sub] = S_total
                    chunks = []
                    off = 0
                    while off < s_:
                        k_ = min(P, s_ - off)
                        chunks.append((T, S_total + off, k_))
                        T += 1
                        off += k_
                    S_total += s_
                    went.append((gg, bounds[sub], bounds[sub + 1], chunks))
            if went:
                wplans.append((w, went))
        plan.append((t0_pair, soff_pair, wplans))

    # slot assignment: rank of each item within its (core, g, w, sub) group
    order = np.argsort(key, kind="stable")
    key_s = key[order]
    grp_first = np.concatenate([[0], np.cumsum(np.bincount(key_s))[:-1]])
    j = np.arange(key_s.shape[0]) - grp_first[key_s]

    g_s = g_v[order]
    w_s = w_v[order]
    sub_s = sub_v[order]
    col = slot_start[g_s, w_s, sub_s] + j  # column within the core's feats
    tcol = t_start[g_s, w_s, sub_s] + j // P
    core_s = core_v[order]
    rc_s = rc_local[order]
    feat_s = feat_v[order]

    feats_cores = []
    rc_cores = []
    for k in range(NCORES):
        m = core_s == k
        fa = np.zeros((F, S_total), np.float32)
        fa[:, col[m]] = feat_s[m].T
        ra = np.full((P, T), -1, np.float32)
        ra[j[m] % P, tcol[m]] = rc_s[m]
        feats_cores.append(fa.astype(np.float16))
        rc_cores.append(ra)

    W96 = np.concatenate(
        [
            np.asarray(inputs["W_edge"], np.float32),
            np.asarray(inputs["W_pair"], np.float32),
            np.asarray(inputs["W_node"], np.float32),
            np.asarray(inputs["W_loop"], np.float32),
        ],
        axis=0,
    ).astype(np.float16)

    mask = np.zeros((B, N), bool)
    nv = (pos >= 0) & (pos < N) & (batch >= 0) & (batch < B)
    mask[batch[nv], pos[nv]] = True

    live_w = sorted({w for (_, _, wplans) in plan for (w, _) in wplans})
    return feats_cores, rc_cores, W96, plan, (T, S_total), live_w, cw, mask


def _build_program(plan, T, live_w, cw):
    """Build + compile the (SPMD-uniform) Bass program."""
    T, S_total = T
    nc = bacc.Bacc("TRN2", num_devices=NCORES)

    rblk = WIN // cw  # output rows covered by one window
    w_hi = max(live_w) + 1 if live_w else 1
    rl = w_hi * rblk  # live output rows (r >= rl is structurally zero)
    live_cells = w_hi * WIN

    feats_d = nc.dram_tensor("feats", [F, S_total], _f16, kind="ExternalInput")
    rc_d = nc.dram_tensor("rc", [P, T], _f32, kind="ExternalInput")
    w96_d = nc.dram_tensor("w96", [F, H], _f16, kind="ExternalInput")
    # only the live [r < rl, c < cw] block, in fp16 (the values carry bf16
    # rounding already; fp16 adds ~5e-4 rel) — host converts and zero-pads
    out_d = nc.dram_tensor("out", [GPC, H, rl, cw], _f16, kind="ExternalOutput")
    out_v = out_d.ap().rearrange("g h r c -> (g h) (r c)")

    with tile.TileContext(nc) as tc, ExitStack() as ctx:
        const = ctx.enter_context(tc.tile_pool(name="const", bufs=1))
        v_p = ctx.enter_context(tc.tile_pool(name="v", bufs=4))
        oh_p = ctx.enter_context(tc.tile_pool(name="oh", bufs=64))
        pv_p = ctx.enter_context(tc.tile_pool(name="pv", bufs=2, space="PSUM"))
        pw_p = ctx.enter_context(tc.tile_pool(name="pw", bufs=3, space="PSUM"))

        iota_t = const.tile([P, WIN], dtype=_i16)
        nc.gpsimd.iota(iota_t[:], pattern=[[1, WIN]], base=0, channel_multiplier=0)

        # PE HAM warm-up: the tensor engine idles during the input-DMA ramp
        # and would otherwise start the real matmuls at the cold 1.2 GHz
        # p-state; burn the idle time on dummy matmuls so the stream is warm.
        warm_t = const.tile([P, WIN], dtype=_f16)
        nc.vector.memset(warm_t[:], 0.0)
        for wu in range(8):
            pvw = pv_p.tile([P, 8 * H], dtype=_f32, tag="pv", name="pvw")
            nc.tensor.matmul(
                out=pvw[:8, :],
                lhsT=warm_t[:, :8],
                rhs=warm_t[:],
                start=True,
                stop=True,
            )
        rc_t = const.tile([P, T], dtype=_f32)
        nc.scalar.dma_start(out=rc_t[:], in_=rc_d.ap())
        w96_t = const.tile([F, H], dtype=_f16)
        nc.scalar.dma_start(out=w96_t[:], in_=w96_d.ap())

        # slabs hold the live cells in compact [r*cw + c] layout; zeroed
        # once — pairs only rewrite blocks of windows that have items, and
        # windows with no items anywhere stay zero.
        slabs = [
            const.tile([P, live_cells], dtype=_f16, tag=f"slab{i}", name=f"slab{i}")
            for i in range(2)
        ]
        gap_w = [w for w in range(w_hi) if w not in set(live_w)]
        for sl in slabs:
            for w in gap_w:
                nc.gpsimd.memset(sl[:, w * WIN : (w + 1) * WIN], 0.0)
        # batch window-block DMAs to >= 1 MiB
        wgrp = 2

        # prefetch all feats up front on the ACT HWDGE queue so the input
        # loads never sit behind output DMAs in a FIFO
        # one resident feats tile; one slice-DMA per pair keeps HWDGE issue
        # overhead low while the first pair still lands early
        feats_all = const.tile([F, S_total], dtype=_f16)
        pair_chunks_all = []
        for pair in range(GPC // 2):
            t0_pair, soff_pair, wplans = plan[pair]
            chunks = [
                ch for (_, went) in wplans for (_, _, _, cl) in went for ch in cl
            ]
            chunks.sort()  # by chunk id == consumption order
            nslots = sum(k_ for (_, _, k_) in chunks)
            pair_chunks_all.append((chunks, nslots))
            if nslots:
                if pair == 0:
                    # split the first pair so its m1 matmuls start sooner
                    h1 = (nslots + 1) // 2
                    nc.scalar.dma_start(
                        out=feats_all[:, soff_pair : soff_pair + h1],
                        in_=feats_d.ap()[:, soff_pair : soff_pair + h1],
                    )
                    nc.scalar.dma_start(
                        out=feats_all[:, soff_pair + h1 : soff_pair + nslots],
                        in_=feats_d.ap()[:, soff_pair + h1 : soff_pair + nslots],
                    )
                else:
                    nc.scalar.dma_start(
                        out=feats_all[:, soff_pair : soff_pair + nslots],
                        in_=feats_d.ap()[:, soff_pair : soff_pair + nslots],
                    )

        for pair in range(GPC // 2):
            t0_pair, soff_pair, wplans = plan[pair]
            slab = slabs[pair % 2]
            chunks, nslots = pair_chunks_all[pair]
            nch = len(chunks)

            v_t = v_p.tile([P, max(nch, 1) * H], dtype=_f16, tag="v")
            if nch:
                # value matmuls, 8 chunks per PSUM drain
                for qi, q in enumerate(range(0, nch, 8)):
                    qn = min(8, nch - q)
                    pv = pv_p.tile([P, 8 * H], dtype=_f32)
                    for jj in range(qn):
                        (t, soff, k_) = chunks[q + jj]
                        nc.tensor.matmul(
                            out=pv[:k_, jj * H : (jj + 1) * H],
                            lhsT=feats_all[:, soff : soff + k_],
                            rhs=w96_t[:],
                            start=True,
                            stop=True,
                        )
                    nc.scalar.copy(
                        out=v_t[:, q * H : (q + qn) * H], in_=pv[:, : qn * H]
                    )

            # one-hots are built lazily (tensor_scalar: int16 iota vs f32
            # per-partition scalar -> bf16, hits the DVE 4x mode), in the
            # exact order the scatter matmuls consume them; some go to the
            # otherwise-idle GpSimd engine
            oh_n = [0]

            def oh_rhs(t, lo, hi, k_):
                oh = oh_p.tile([P, hi - lo], dtype=_f16, tag="oh", name="oh")
                eng = nc.gpsimd if oh_n[0] % 4 == 3 else nc.vector
                oh_n[0] += 1
                eng.tensor_scalar(
                    out=oh[:k_],
                    in0=iota_t[:k_, lo:hi],
                    scalar1=rc_t[:k_, t : t + 1],
                    scalar2=None,
                    op0=mybir.AluOpType.is_equal,
                )
                return oh[:k_]

            live_by_w = dict(wplans)
            # process windows in adjacent groups sharing one multi-bank PSUM
            # tile, so each slab copy covers the whole group
            for wi in range(0, len(live_w), 2):
                wgroup = live_w[wi : wi + 2]
                if wgroup != list(range(wgroup[0], wgroup[0] + len(wgroup))):
                    wgroup = wgroup[:1]  # non-adjacent: fall back to single
                ng = len(wgroup)
                ps = pw_p.tile([P, ng * WIN], dtype=_f32, tag="ps", name="ps")
                for wj, w in enumerate(wgroup):
                    off = wj * WIN
                    went = live_by_w.get(w, [])
                    # zero PSUM column ranges no matmul will write
                    # (vector engine only — GpSimd can't touch PSUM)
                    for gg in range(2):
                        covered = sorted(
                            (lo, hi) for (g2, lo, hi, _) in went if g2 == gg
                        )
                        pos_ = 0
                        for (lo, hi) in covered + [(WIN, WIN)]:
                            if lo > pos_:
                                nc.vector.memset(
                                    ps[gg * H : (gg + 1) * H, off + pos_ : off + lo],
                                    0.0,
                                )
                            pos_ = max(pos_, hi)
                    for (gg, lo, hi, cl) in went:
                        for cc, (t, _, k_) in enumerate(cl):
                            lt = t - t0_pair
                            nc.tensor.matmul(
                                out=ps[gg * H : (gg + 1) * H, off + lo : off + hi],
                                lhsT=v_t[:k_, lt * H : (lt + 1) * H],
                                rhs=oh_rhs(t, lo, hi, k_),
                                start=(cc == 0),
                                stop=(cc == len(cl) - 1),
                            )
                w0 = wgroup[0]
                if (wi // 2) % 2 == 1:
                    nc.vector.tensor_copy(
                        out=slab[:, w0 * WIN : (w0 + ng) * WIN], in_=ps[:]
                    )
                else:
                    nc.scalar.copy(
                        out=slab[:, w0 * WIN : (w0 + ng) * WIN], in_=ps[:]
                    )

            rows = slice(pair * P, (pair + 1) * P)
            for w0 in range(0, w_hi, wgrp):
                c0 = w0 * WIN
                c1 = min((w0 + wgrp) * WIN, live_cells)
                nc.sync.dma_start(out=out_v[rows, c0:c1], in_=slab[:, c0:c1])

    nc.compile()
    return nc


def _prepare(inputs):
    """Host prep + (cached) program build.  Returns (nc, in_maps, mask)."""
    feats_cores, rc_cores, W96, plan, T, live_w, cw, mask = _host_prep(inputs)

    plan_key = (
        T,
        cw,
        tuple(
            (
                t0,
                s0,
                tuple(
                    (w, tuple((gg, lo, hi, tuple(cl)) for (gg, lo, hi, cl) in went))
                    for (w, went) in wplans
                ),
            )
            for (t0, s0, wplans) in plan
        ),
        tuple(live_w),
    )
    nc = _program_cache.get(plan_key)
    if nc is None:
        nc = _build_program(plan, T, live_w, cw)
        _program_cache[plan_key] = nc

    in_maps = [
        {"feats": feats_cores[k], "rc": rc_cores[k], "w96": W96}
        for k in range(NCORES)
    ]
    return nc, in_maps, mask


def kernel(**inputs):
    nc, in_maps, mask = _prepare(inputs)
    res = run_bass_kernel_spmd(nc, in_maps, core_ids=list(range(NCORES)))
    global _last_results
    _last_results = res
    live = np.concatenate([r["out"] for r in res.results], axis=0)
    _, _, rl, cwc = live.shape
    dense = np.zeros((B, H, N, N), np.float32)
    dense[:, :, :rl, :cwc] = live.astype(np.float32)
    return dense, mask


_last_results = None


# revision 61
# speedup vs baseline: 1.0033x; 1.0033x over previous
"""Trainium2 Bass kernel for nn_DenseInputEncoder (to_dense_adj-style scatter).

Strategy (data-parallel over graphs, 8 graphs per NeuronCore):
  The output dense_pair_h[b, h, r, c] is a mostly-zero dense tensor built from
  ~2k scattered (r, c) cells per graph.  Instead of DMA scatter, each cell
  sub-range of the output is produced DENSE by a TensorE matmul:
      out[h, cell] = sum_items V[item, h] * onehot[item, cell]
  where onehot[item, cell] = (rc_local[item] == iota[cell]) is built by
  tensor_scalar is_equal ops (int16 iota vs f32 per-partition scalar -> fp16,
  DVE 4x mode; a quarter go to the otherwise idle GpSimd).  The matmul
  simultaneously performs the scatter, sums duplicate cells, and fills
  untouched cells with exact zeros.  Item values V = feat96 @ W96 unify the
  edge/pair/node-diagonal encoders (features are placed in disjoint 96-dim
  blocks on the host, so one weight matrix serves all three).

  Work layout: cells are compacted to r*64+c when all c < 64 (true for this
  data), 512-cell PSUM windows are subdivided into ~171-cell sub-ranges so a
  (graph, window, sub) group usually fits one 128-slot chunk, and host-side
  planning pads groups uniformly across cores so one SPMD program serves all
  8 cores.  Graph pairs share [128 x live] fp16 SBUF slabs (partitions 0-63 =
  graph a's h-planes, 64-127 = graph b's) DMA'd out in ~1 MiB blocks as
  windows complete; input DMAs ride the separate ACT HWDGE queue.  Only the
  live [r < rl, c < 64] block is computed on device (fp16); the host converts
  to f32 and pads the structurally-zero remainder.
"""

import numpy as np
import ml_dtypes
from contextlib import ExitStack

import concourse.mybir as mybir
import concourse.tile as tile
from concourse import bacc
from concourse.bass_utils import run_bass_kernel_spmd

B = 64          # graphs
N = 128         # max nodes per graph (dense padding)
H = 64          # hidden dim
NCORES = 8
GPC = B // NCORES  # graphs per core
WIN = 512       # cells per window (one PSUM bank at fp32)
P = 128         # partitions / matmul contraction size
F = 96          # unified feature dim: [edge 32 | pair 16 | node 32 | loop 16]

_f32 = mybir.dt.float32
_bf16 = mybir.dt.bfloat16
_i16 = mybir.dt.int16
_f16 = mybir.dt.float16

_program_cache = {}


def _host_prep(inputs):
    """Index math + feature packing on host (numpy).  Returns per-core input
    arrays, the uniform chunk plan, and the (host-computed) node mask."""
    batch = np.asarray(inputs["batch"]).astype(np.int64)
    edge_index = np.asarray(inputs["edge_index"]).astype(np.int64)
    pair_index = np.asarray(inputs["pair_index"]).astype(np.int64)
    node_x = np.asarray(inputs["node_x"], dtype=np.float32)
    loop_x = np.asarray(inputs["loop_x"], dtype=np.float32)
    edge_attr = np.asarray(inputs["edge_attr"], dtype=np.float32)
    pair_x = np.asarray(inputs["pair_x"], dtype=np.float32)

    NT = batch.shape[0]
    E = edge_index.shape[1]

    # position of each node within its graph (to_dense_batch semantics)
    counts = np.bincount(batch, minlength=B)
    starts = np.concatenate([[0], np.cumsum(counts)[:-1]])
    pos = np.arange(NT, dtype=np.int64) - starts[batch]

    # unified item list: edges, pairs, node-diagonal entries
    e0, e1 = edge_index
    p0, p1 = pair_index
    b_it = np.concatenate([batch[e0], batch[p0], batch])
    r_it = np.concatenate([pos[e0], pos[p0], pos])
    c_it = np.concatenate([pos[e1], pos[p1], pos])
    n_items = b_it.shape[0]

    feat = np.zeros((n_items, F), np.float32)
    feat[:E, 0:32] = edge_attr
    feat[E : 2 * E, 32:48] = pair_x
    feat[2 * E :, 48:80] = node_x
    feat[2 * E :, 80:96] = loop_x

    # out-of-bounds scatter indices are dropped (jax .at[] default)
    valid = (r_it >= 0) & (r_it < N) & (c_it >= 0) & (c_it < N) & (b_it >= 0) & (b_it < B)
    b_v, r_v, c_v = b_it[valid], r_it[valid], c_it[valid]
    feat_v = feat[valid]

    # column compaction: if all c < 64, use a r*64+c cell space (halves the
    # number of scatter windows); the slab copy re-expands.
    cw = 64 if (c_v.size == 0 or c_v.max() < 64) else N
    nwin = (N * cw) // WIN
    cell = r_v * cw + c_v
    w_v = cell // WIN
    rc_local = (cell % WIN).astype(np.float32)
    core_v = b_v // GPC
    g_v = b_v % GPC

    # Subdivide each 512-cell window into SUB cell sub-ranges so that one
    # (g, w, sub) group usually fits a single 128-slot chunk — narrower
    # sub-ranges mean narrower scatter matmuls and one-hot compares.  Pick
    # SUB by a PE-cost proxy over the candidates.
    in_w = (cell % WIN).astype(np.int64)
    best = None
    for sub_try in (1, 2, 3, 4):
        bounds = [round(s * WIN / sub_try) for s in range(sub_try + 1)]
        widths = np.diff(bounds)
        sub_v_t = np.minimum(
            np.searchsorted(bounds, in_w, side="right") - 1, sub_try - 1
        )
        key_t = ((core_v * GPC + g_v) * nwin + w_v) * sub_try + sub_v_t
        cnt_t = np.bincount(
            key_t, minlength=NCORES * GPC * nwin * sub_try
        ).reshape(NCORES, GPC, nwin, sub_try)
        C_t = -(-cnt_t.max(axis=0) // P)  # [GPC, nwin, sub]
        pe_cost = int((C_t * (P + widths[None, None, :])).sum())
        if best is None or pe_cost < best[0]:
            best = (pe_cost, sub_try, bounds, sub_v_t, key_t, C_t)
    _, SUB, bounds, sub_v, key, C_gws = best

    # chunk table: chunk ids ordered by consumption — (pair, w, graph, sub,
    # cc) — so the one-hot builds and matmuls stream in the same order.
    # Chunks carry exact slot counts (K <= 128): feats slots are packed, not
    # padded to 128 per chunk.
    S_gws = np.bincount(key, minlength=NCORES * GPC * nwin * SUB).reshape(
        NCORES, GPC, nwin, SUB
    ).max(axis=0)
    t_start = np.zeros((GPC, nwin, SUB), np.int64)
    slot_start = np.zeros((GPC, nwin, SUB), np.int64)
    plan = []  # per pair: (t0, soff0, [(w, [(gg, lo, hi, [(t, soff, K)...])])])
    T = 0
    S_total = 0
    for pair in range(GPC // 2):
        t0_pair = T
        soff_pair = S_total
        wplans = []
        for w in range(nwin):
            went = []
            for gg in range(2):
                g = 2 * pair + gg
                for sub in range(SUB):
                    s_ = int(S_gws[g, w, sub])
                    if s_ == 0:
                        continue
                    t_start[g, w, sub] = T
                    slot_start[g, w, sub] = S_total
                    chunks = []
                    off = 0
                    while off < s_:
                        k_ = min(P, s_ - off)
                        chunks.append((T, S_total + off, k_))
                        T += 1
                        off += k_
                    S_total += s_
                    went.append((gg, bounds[sub], bounds[sub + 1], chunks))
            if went:
                wplans.append((w, went))
        plan.append((t0_pair, soff_pair, wplans))

    # slot assignment: rank of each item within its (core, g, w, sub) group
    order = np.argsort(key, kind="stable")
    key_s = key[order]
    grp_first = np.concatenate([[0], np.cumsum(np.bincount(key_s))[:-1]])
    j = np.arange(key_s.shape[0]) - grp_first[key_s]

    g_s = g_v[order]
    w_s = w_v[order]
    sub_s = sub_v[order]
    col = slot_start[g_s, w_s, sub_s] + j  # column within the core's feats
    tcol = t_start[g_s, w_s, sub_s] + j // P
    core_s = core_v[order]
    rc_s = rc_local[order]
    feat_s = feat_v[order]

    feats_cores = []
    rc_cores = []
    for k in range(NCORES):
        m = core_s == k
        fa = np.zeros((F, S_total), np.float32)
        fa[:, col[m]] = feat_s[m].T
        ra = np.full((P, T), -1, np.float32)
        ra[j[m] % P, tcol[m]] = rc_s[m]
        feats_cores.append(fa.astype(np.float16))
        rc_cores.append(ra)

    W96 = np.concatenate(
        [
            np.asarray(inputs["W_edge"], np.float32),
            np.asarray(inputs["W_pair"], np.float32),
            np.asarray(inputs["W_node"], np.float32),
            np.asarray(inputs["W_loop"], np.float32),
        ],
        axis=0,
    ).astype(np.float16)

    mask = np.zeros((B, N), bool)
    nv = (pos >= 0) & (pos < N) & (batch >= 0) & (batch < B)
    mask[batch[nv], pos[nv]] = True

    live_w = sorted({w for (_, _, wplans) in plan for (w, _) in wplans})
    return feats_cores, rc_cores, W96, plan, (T, S_total), live_w, cw, mask


def _build_program(plan, T, live_w, cw):
    """Build + compile the (SPMD-uniform) Bass program."""
    T, S_total = T
    nc = bacc.Bacc("TRN2", num_devices=NCORES)

    rblk = WIN // cw  # output rows covered by one window
    w_hi = max(live_w) + 1 if live_w else 1
    rl = w_hi * rblk  # live output rows (r >= rl is structurally zero)
    live_cells = w_hi * WIN

    feats_d = nc.dram_tensor("feats", [F, S_total], _f16, kind="ExternalInput")
    rc_d = nc.dram_tensor("rc", [P, T], _f32, kind="ExternalInput")
    w96_d = nc.dram_tensor("w96", [F, H], _f16, kind="ExternalInput")
    # only the live [r < rl, c < cw] block, in fp16 (the values carry bf16
    # rounding already; fp16 adds ~5e-4 rel) — host converts and zero-pads
    out_d = nc.dram_tensor("out", [GPC, H, rl, cw], _f16, kind="ExternalOutput")
    out_v = out_d.ap().rearrange("g h r c -> (g h) (r c)")

    with tile.TileContext(nc) as tc, ExitStack() as ctx:
        const = ctx.enter_context(tc.tile_pool(name="const", bufs=1))
        v_p = ctx.enter_context(tc.tile_pool(name="v", bufs=4))
        oh_p = ctx.enter_context(tc.tile_pool(name="oh", bufs=64))
        pv_p = ctx.enter_context(tc.tile_pool(name="pv", bufs=2, space="PSUM"))
        pw_p = ctx.enter_context(tc.tile_pool(name="pw", bufs=3, space="PSUM"))

        iota_t = const.tile([P, WIN], dtype=_i16)
        nc.gpsimd.iota(iota_t[:], pattern=[[1, WIN]], base=0, channel_multiplier=0)

        # PE HAM warm-up: the tensor engine idles during the input-DMA ramp
        # and would otherwise start the real matmuls at the cold 1.2 GHz
        # p-state; burn the idle time on dummy matmuls so the stream is warm.
        warm_t = const.tile([P, WIN], dtype=_f16)
        nc.vector.memset(warm_t[:], 0.0)
        for wu in range(8):
            pvw = pv_p.tile([P, 8 * H], dtype=_f32, tag="pv", name="pvw")
            nc.tensor.matmul(
                out=pvw[:8, :],
                lhsT=warm_t[:, :8],
                rhs=warm_t[:],
                start=True,
                stop=True,
            )
        rc_t = const.tile([P, T], dtype=_f32)
        nc.scalar.dma_start(out=rc_t[:], in_=rc_d.ap())
        w96_t = const.tile([F, H], dtype=_f16)
        nc.sync.dma_start(out=w96_t[:], in_=w96_d.ap())

        # slabs hold the live cells in compact [r*cw + c] layout; zeroed
        # once — pairs only rewrite blocks of windows that have items, and
        # windows with no items anywhere stay zero.
        slabs = [
            const.tile([P, live_cells], dtype=_f16, tag=f"slab{i}", name=f"slab{i}")
            for i in range(2)
        ]
        gap_w = [w for w in range(w_hi) if w not in set(live_w)]
        for sl in slabs:
            for w in gap_w:
                nc.gpsimd.memset(sl[:, w * WIN : (w + 1) * WIN], 0.0)
        # batch window-block DMAs to >= 1 MiB
        wgrp = 2

        # prefetch all feats up front on the ACT HWDGE queue so the input
        # loads never sit behind output DMAs in a FIFO
        # one resident feats tile; one slice-DMA per pair keeps HWDGE issue
        # overhead low while the first pair still lands early
        feats_all = const.tile([F, S_total], dtype=_f16)
        pair_chunks_all = []
        for pair in range(GPC // 2):
            t0_pair, soff_pair, wplans = plan[pair]
            chunks = [
                ch for (_, went) in wplans for (_, _, _, cl) in went for ch in cl
            ]
            chunks.sort()  # by chunk id == consumption order
            nslots = sum(k_ for (_, _, k_) in chunks)
            pair_chunks_all.append((chunks, nslots))
            if nslots:
                if pair == 0:
                    # split the first pair so its m1 matmuls start sooner
                    h1 = (nslots + 1) // 2
                    nc.scalar.dma_start(
                        out=feats_all[:, soff_pair : soff_pair + h1],
                        in_=feats_d.ap()[:, soff_pair : soff_pair + h1],
                    )
                    nc.scalar.dma_start(
                        out=feats_all[:, soff_pair + h1 : soff_pair + nslots],
                        in_=feats_d.ap()[:, soff_pair + h1 : soff_pair + nslots],
                    )
                else:
                    nc.scalar.dma_start(
                        out=feats_all[:, soff_pair : soff_pair + nslots],
                        in_=feats_d.ap()[:, soff_pair : soff_pair + nslots],
                    )

        for pair in range(GPC // 2):
            t0_pair, soff_pair, wplans = plan[pair]
            slab = slabs[pair % 2]
            chunks, nslots = pair_chunks_all[pair]
            nch = len(chunks)

            v_t = v_p.tile([P, max(nch, 1) * H], dtype=_f16, tag="v")
            if nch:
                # value matmuls, 8 chunks per PSUM drain
                for qi, q in enumerate(range(0, nch, 8)):
                    qn = min(8, nch - q)
                    pv = pv_p.tile([P, 8 * H], dtype=_f32)
                    for jj in range(qn):
                        (t, soff, k_) = chunks[q + jj]
                        nc.tensor.matmul(
                            out=pv[:k_, jj * H : (jj + 1) * H],
                            lhsT=feats_all[:, soff : soff + k_],
                            rhs=w96_t[:],
                            start=True,
                            stop=True,
                        )
                    nc.scalar.copy(
                        out=v_t[:, q * H : (q + qn) * H], in_=pv[:, : qn * H]
                    )

            # one-hots are built lazily (tensor_scalar: int16 iota vs f32
            # per-partition scalar -> bf16, hits the DVE 4x mode), in the
            # exact order the scatter matmuls consume them; some go to the
            # otherwise-idle GpSimd engine
            oh_n = [0]

            def oh_rhs(t, lo, hi, k_):
                oh = oh_p.tile([P, hi - lo], dtype=_f16, tag="oh", name="oh")
                eng = nc.gpsimd if oh_n[0] % 4 == 3 else nc.vector
                oh_n[0] += 1
                eng.tensor_scalar(
                    out=oh[:k_],
                    in0=iota_t[:k_, lo:hi],
                    scalar1=rc_t[:k_, t : t + 1],
                    scalar2=None,
                    op0=mybir.AluOpType.is_equal,
                )
                return oh[:k_]

            live_by_w = dict(wplans)
            # process windows in adjacent groups sharing one multi-bank PSUM
            # tile, so each slab copy covers the whole group
            for wi in range(0, len(live_w), 2):
                wgroup = live_w[wi : wi + 2]
                if wgroup != list(range(wgroup[0], wgroup[0] + len(wgroup))):
                    wgroup = wgroup[:1]  # non-adjacent: fall back to single
                ng = len(wgroup)
                ps = pw_p.tile([P, ng * WIN], dtype=_f32, tag="ps", name="ps")
                for wj, w in enumerate(wgroup):
                    off = wj * WIN
                    went = live_by_w.get(w, [])
                    # zero PSUM column ranges no matmul will write
                    # (vector engine only — GpSimd can't touch PSUM)
                    for gg in range(2):
                        covered = sorted(
                            (lo, hi) for (g2, lo, hi, _) in went if g2 == gg
                        )
                        pos_ = 0
                        for (lo, hi) in covered + [(WIN, WIN)]:
                            if lo > pos_:
                                nc.vector.memset(
                                    ps[gg * H : (gg + 1) * H, off + pos_ : off + lo],
                                    0.0,
                                )
                            pos_ = max(pos_, hi)
                    for (gg, lo, hi, cl) in went:
                        for cc, (t, _, k_) in enumerate(cl):
                            lt = t - t0_pair
                            nc.tensor.matmul(
                                out=ps[gg * H : (gg + 1) * H, off + lo : off + hi],
                                lhsT=v_t[:k_, lt * H : (lt + 1) * H],
                                rhs=oh_rhs(t, lo, hi, k_),
                                start=(cc == 0),
                                stop=(cc == len(cl) - 1),
                            )
                w0 = wgroup[0]
                if (wi // 2) % 2 == 1:
                    nc.vector.tensor_copy(
                        out=slab[:, w0 * WIN : (w0 + ng) * WIN], in_=ps[:]
                    )
                else:
                    nc.scalar.copy(
                        out=slab[:, w0 * WIN : (w0 + ng) * WIN], in_=ps[:]
                    )

            rows = slice(pair * P, (pair + 1) * P)
            for w0 in range(0, w_hi, wgrp):
                c0 = w0 * WIN
                c1 = min((w0 + wgrp) * WIN, live_cells)
                nc.sync.dma_start(out=out_v[rows, c0:c1], in_=slab[:, c0:c1])

    nc.compile()
    return nc


def _prepare(inputs):
    """Host prep + (cached) program build.  Returns (nc, in_maps, mask)."""
    feats_cores, rc_cores, W96, plan, T, live_w, cw, mask = _host_prep(inputs)

    plan_key = (
        T,
        cw,
        tuple(
            (
                t0,
                s0,
                tuple(
                    (w, tuple((gg, lo, hi, tuple(cl)) for (gg, lo, hi, cl) in went))
                    for (w, went) in wplans
                ),
            )
            for (t0, s0, wplans) in plan
        ),
        tuple(live_w),
    )
    nc = _program_cache.get(plan_key)
    if nc is None:
        nc = _build_program(plan, T, live_w, cw)
        _program_cache[plan_key] = nc

    in_maps = [
        {"feats": feats_cores[k], "rc": rc_cores[k], "w96": W96}
        for k in range(NCORES)
    ]
    return nc, in_maps, mask


def kernel(**inputs):
    nc, in_maps, mask = _prepare(inputs)
    res = run_bass_kernel_spmd(nc, in_maps, core_ids=list(range(NCORES)))
    global _last_results
    _last_results = res
    live = np.concatenate([r["out"] for r in res.results], axis=0)
    _, _, rl, cwc = live.shape
    dense = np.zeros((B, H, N, N), np.float32)
    dense[:, :, :rl, :cwc] = live.astype(np.float32)
    return dense, mask


_last_results = None


# revision 62
# speedup vs baseline: 1.0203x; 1.0170x over previous
"""Trainium2 Bass kernel for nn_DenseInputEncoder (to_dense_adj-style scatter).

Strategy (data-parallel over graphs, 8 graphs per NeuronCore):
  The output dense_pair_h[b, h, r, c] is a mostly-zero dense tensor built from
  ~2k scattered (r, c) cells per graph.  Instead of DMA scatter, each cell
  sub-range of the output is produced DENSE by a TensorE matmul:
      out[h, cell] = sum_items V[item, h] * onehot[item, cell]
  where onehot[item, cell] = (rc_local[item] == iota[cell]) is built by
  tensor_scalar is_equal ops (int16 iota vs f32 per-partition scalar -> fp16,
  DVE 4x mode; a quarter go to the otherwise idle GpSimd).  The matmul
  simultaneously performs the scatter, sums duplicate cells, and fills
  untouched cells with exact zeros.  Item values V = feat96 @ W96 unify the
  edge/pair/node-diagonal encoders (features are placed in disjoint 96-dim
  blocks on the host, so one weight matrix serves all three).

  Work layout: cells are compacted to r*64+c when all c < 64 (true for this
  data), 512-cell PSUM windows are subdivided into ~171-cell sub-ranges so a
  (graph, window, sub) group usually fits one 128-slot chunk, and host-side
  planning pads groups uniformly across cores so one SPMD program serves all
  8 cores.  Graph pairs share [128 x live] fp16 SBUF slabs (partitions 0-63 =
  graph a's h-planes, 64-127 = graph b's) DMA'd out in ~1 MiB blocks as
  windows complete; input DMAs ride the separate ACT HWDGE queue.  Only the
  live [r < rl, c < 64] block is computed on device (fp16); the host converts
  to f32 and pads the structurally-zero remainder.
"""

import numpy as np
import ml_dtypes
from contextlib import ExitStack

import concourse.mybir as mybir
import concourse.tile as tile
from concourse import bacc
from concourse.bass_utils import run_bass_kernel_spmd

B = 64          # graphs
N = 128         # max nodes per graph (dense padding)
H = 64          # hidden dim
NCORES = 8
GPC = B // NCORES  # graphs per core
WIN = 512       # cells per window (one PSUM bank at fp32)
P = 128         # partitions / matmul contraction size
F = 96          # unified feature dim: [edge 32 | pair 16 | node 32 | loop 16]

_f32 = mybir.dt.float32
_bf16 = mybir.dt.bfloat16
_i16 = mybir.dt.int16
_f16 = mybir.dt.float16

_program_cache = {}


def _host_prep(inputs):
    """Index math + feature packing on host (numpy).  Returns per-core input
    arrays, the uniform chunk plan, and the (host-computed) node mask."""
    batch = np.asarray(inputs["batch"]).astype(np.int64)
    edge_index = np.asarray(inputs["edge_index"]).astype(np.int64)
    pair_index = np.asarray(inputs["pair_index"]).astype(np.int64)
    node_x = np.asarray(inputs["node_x"], dtype=np.float32)
    loop_x = np.asarray(inputs["loop_x"], dtype=np.float32)
    edge_attr = np.asarray(inputs["edge_attr"], dtype=np.float32)
    pair_x = np.asarray(inputs["pair_x"], dtype=np.float32)

    NT = batch.shape[0]
    E = edge_index.shape[1]

    # position of each node within its graph (to_dense_batch semantics)
    counts = np.bincount(batch, minlength=B)
    starts = np.concatenate([[0], np.cumsum(counts)[:-1]])
    pos = np.arange(NT, dtype=np.int64) - starts[batch]

    # unified item list: edges, pairs, node-diagonal entries
    e0, e1 = edge_index
    p0, p1 = pair_index
    b_it = np.concatenate([batch[e0], batch[p0], batch])
    r_it = np.concatenate([pos[e0], pos[p0], pos])
    c_it = np.concatenate([pos[e1], pos[p1], pos])
    n_items = b_it.shape[0]

    feat = np.zeros((n_items, F), np.float32)
    feat[:E, 0:32] = edge_attr
    feat[E : 2 * E, 32:48] = pair_x
    feat[2 * E :, 48:80] = node_x
    feat[2 * E :, 80:96] = loop_x

    # out-of-bounds scatter indices are dropped (jax .at[] default)
    valid = (r_it >= 0) & (r_it < N) & (c_it >= 0) & (c_it < N) & (b_it >= 0) & (b_it < B)
    b_v, r_v, c_v = b_it[valid], r_it[valid], c_it[valid]
    feat_v = feat[valid]

    # column compaction: if all c < 64, use a r*64+c cell space (halves the
    # number of scatter windows); the slab copy re-expands.
    cw = 64 if (c_v.size == 0 or c_v.max() < 64) else N
    nwin = (N * cw) // WIN
    cell = r_v * cw + c_v
    w_v = cell // WIN
    rc_local = (cell % WIN).astype(np.float32)
    core_v = b_v // GPC
    g_v = b_v % GPC

    # Subdivide each 512-cell window into SUB cell sub-ranges so that one
    # (g, w, sub) group usually fits a single 128-slot chunk — narrower
    # sub-ranges mean narrower scatter matmuls and one-hot compares.  Pick
    # SUB by a PE-cost proxy over the candidates.
    in_w = (cell % WIN).astype(np.int64)
    best = None
    for sub_try in (1, 2, 3, 4):
        bounds = [round(s * WIN / sub_try) for s in range(sub_try + 1)]
        widths = np.diff(bounds)
        sub_v_t = np.minimum(
            np.searchsorted(bounds, in_w, side="right") - 1, sub_try - 1
        )
        key_t = ((core_v * GPC + g_v) * nwin + w_v) * sub_try + sub_v_t
        cnt_t = np.bincount(
            key_t, minlength=NCORES * GPC * nwin * sub_try
        ).reshape(NCORES, GPC, nwin, sub_try)
        C_t = -(-cnt_t.max(axis=0) // P)  # [GPC, nwin, sub]
        pe_cost = int((C_t * (P + widths[None, None, :])).sum())
        if best is None or pe_cost < best[0]:
            best = (pe_cost, sub_try, bounds, sub_v_t, key_t, C_t)
    _, SUB, bounds, sub_v, key, C_gws = best

    # chunk table: chunk ids ordered by consumption — (pair, w, graph, sub,
    # cc) — so the one-hot builds and matmuls stream in the same order.
    # Chunks carry exact slot counts (K <= 128): feats slots are packed, not
    # padded to 128 per chunk.
    S_gws = np.bincount(key, minlength=NCORES * GPC * nwin * SUB).reshape(
        NCORES, GPC, nwin, SUB
    ).max(axis=0)
    t_start = np.zeros((GPC, nwin, SUB), np.int64)
    slot_start = np.zeros((GPC, nwin, SUB), np.int64)
    plan = []  # per pair: (t0, soff0, [(w, [(gg, lo, hi, [(t, soff, K)...])])])
    T = 0
    S_total = 0
    for pair in range(GPC // 2):
        t0_pair = T
        soff_pair = S_total
        wplans = []
        for w in range(nwin):
            went = []
            for gg in range(2):
                g = 2 * pair + gg
                for sub in range(SUB):
                    s_ = int(S_gws[g, w, sub])
                    if s_ == 0:
                        continue
                    t_start[g, w, sub] = T
                    slot_start[g, w, sub] = S_total
                    chunks = []
                    off = 0
                    while off < s_:
                        k_ = min(P, s_ - off)
                        chunks.append((T, S_total + off, k_))
                        T += 1
                        off += k_
                    S_total += s_
                    went.append((gg, bounds[sub], bounds[sub + 1], chunks))
            if went:
                wplans.append((w, went))
        plan.append((t0_pair, soff_pair, wplans))

    # slot assignment: rank of each item within its (core, g, w, sub) group
    order = np.argsort(key, kind="stable")
    key_s = key[order]
    grp_first = np.concatenate([[0], np.cumsum(np.bincount(key_s))[:-1]])
    j = np.arange(key_s.shape[0]) - grp_first[key_s]

    g_s = g_v[order]
    w_s = w_v[order]
    sub_s = sub_v[order]
    col = slot_start[g_s, w_s, sub_s] + j  # column within the core's feats
    tcol = t_start[g_s, w_s, sub_s] + j // P
    core_s = core_v[order]
    rc_s = rc_local[order]
    feat_s = feat_v[order]

    feats_cores = []
    rc_cores = []
    for k in range(NCORES):
        m = core_s == k
        fa = np.zeros((F, S_total), np.float32)
        fa[:, col[m]] = feat_s[m].T
        ra = np.full((P, T), -1, np.float32)
        ra[j[m] % P, tcol[m]] = rc_s[m]
        feats_cores.append(fa.astype(np.float16))
        rc_cores.append(ra)

    W96 = np.concatenate(
        [
            np.asarray(inputs["W_edge"], np.float32),
            np.asarray(inputs["W_pair"], np.float32),
            np.asarray(inputs["W_node"], np.float32),
            np.asarray(inputs["W_loop"], np.float32),
        ],
        axis=0,
    ).astype(np.float16)

    mask = np.zeros((B, N), bool)
    nv = (pos >= 0) & (pos < N) & (batch >= 0) & (batch < B)
    mask[batch[nv], pos[nv]] = True

    live_w = sorted({w for (_, _, wplans) in plan for (w, _) in wplans})
    return feats_cores, rc_cores, W96, plan, (T, S_total), live_w, cw, mask


def _build_program(plan, T, live_w, cw):
    """Build + compile the (SPMD-uniform) Bass program."""
    T, S_total = T
    nc = bacc.Bacc("TRN2", num_devices=NCORES)

    rblk = WIN // cw  # output rows covered by one window
    w_hi = max(live_w) + 1 if live_w else 1
    rl = w_hi * rblk  # live output rows (r >= rl is structurally zero)
    live_cells = w_hi * WIN

    feats_d = nc.dram_tensor("feats", [F, S_total], _f16, kind="ExternalInput")
    rc_d = nc.dram_tensor("rc", [P, T], _f32, kind="ExternalInput")
    w96_d = nc.dram_tensor("w96", [F, H], _f16, kind="ExternalInput")
    # only the live [r < rl, c < cw] block, in fp16 (the values carry bf16
    # rounding already; fp16 adds ~5e-4 rel) — host converts and zero-pads
    out_d = nc.dram_tensor("out", [GPC, H, rl, cw], _f16, kind="ExternalOutput")
    out_v = out_d.ap().rearrange("g h r c -> (g h) (r c)")

    with tile.TileContext(nc) as tc, ExitStack() as ctx:
        const = ctx.enter_context(tc.tile_pool(name="const", bufs=1))
        v_p = ctx.enter_context(tc.tile_pool(name="v", bufs=4))
        oh_p = ctx.enter_context(tc.tile_pool(name="oh", bufs=64))
        pv_p = ctx.enter_context(tc.tile_pool(name="pv", bufs=2, space="PSUM"))
        pw_p = ctx.enter_context(tc.tile_pool(name="pw", bufs=3, space="PSUM"))

        iota_t = const.tile([P, WIN], dtype=_i16)
        nc.gpsimd.iota(iota_t[:], pattern=[[1, WIN]], base=0, channel_multiplier=0)

        # PE HAM warm-up: the tensor engine idles during the input-DMA ramp
        # and would otherwise start the real matmuls at the cold 1.2 GHz
        # p-state; burn the idle time on dummy matmuls so the stream is warm.
        warm_t = const.tile([P, WIN], dtype=_f16)
        nc.vector.memset(warm_t[:], 0.0)
        for wu in range(8):
            pvw = pv_p.tile([P, 8 * H], dtype=_f32, tag="pv", name="pvw")
            nc.tensor.matmul(
                out=pvw[:8, :],
                lhsT=warm_t[:, :8],
                rhs=warm_t[:],
                start=True,
                stop=True,
            )
        rc_t = const.tile([P, T], dtype=_f32)
        nc.sync.dma_start(out=rc_t[:], in_=rc_d.ap())
        w96_t = const.tile([F, H], dtype=_f16)
        nc.sync.dma_start(out=w96_t[:], in_=w96_d.ap())

        # slabs hold the live cells in compact [r*cw + c] layout; zeroed
        # once — pairs only rewrite blocks of windows that have items, and
        # windows with no items anywhere stay zero.
        slabs = [
            const.tile([P, live_cells], dtype=_f16, tag=f"slab{i}", name=f"slab{i}")
            for i in range(2)
        ]
        gap_w = [w for w in range(w_hi) if w not in set(live_w)]
        for sl in slabs:
            for w in gap_w:
                nc.gpsimd.memset(sl[:, w * WIN : (w + 1) * WIN], 0.0)
        # batch window-block DMAs to >= 1 MiB
        wgrp = 2

        # prefetch all feats up front on the ACT HWDGE queue so the input
        # loads never sit behind output DMAs in a FIFO
        # one resident feats tile; one slice-DMA per pair keeps HWDGE issue
        # overhead low while the first pair still lands early
        feats_all = const.tile([F, S_total], dtype=_f16)
        pair_chunks_all = []
        for pair in range(GPC // 2):
            t0_pair, soff_pair, wplans = plan[pair]
            chunks = [
                ch for (_, went) in wplans for (_, _, _, cl) in went for ch in cl
            ]
            chunks.sort()  # by chunk id == consumption order
            nslots = sum(k_ for (_, _, k_) in chunks)
            pair_chunks_all.append((chunks, nslots))
            if nslots:
                if pair == 0:
                    # split the first pair so its m1 matmuls start sooner
                    h1 = (nslots + 1) // 2
                    nc.scalar.dma_start(
                        out=feats_all[:, soff_pair : soff_pair + h1],
                        in_=feats_d.ap()[:, soff_pair : soff_pair + h1],
                    )
                    nc.scalar.dma_start(
                        out=feats_all[:, soff_pair + h1 : soff_pair + nslots],
                        in_=feats_d.ap()[:, soff_pair + h1 : soff_pair + nslots],
                    )
                else:
                    nc.scalar.dma_start(
                        out=feats_all[:, soff_pair : soff_pair + nslots],
                        in_=feats_d.ap()[:, soff_pair : soff_pair + nslots],
                    )

        for pair in range(GPC // 2):
            t0_pair, soff_pair, wplans = plan[pair]
            slab = slabs[pair % 2]
            chunks, nslots = pair_chunks_all[pair]
            nch = len(chunks)

            v_t = v_p.tile([P, max(nch, 1) * H], dtype=_f16, tag="v")
            if nch:
                # value matmuls, 8 chunks per PSUM drain
                for qi, q in enumerate(range(0, nch, 8)):
                    qn = min(8, nch - q)
                    pv = pv_p.tile([P, 8 * H], dtype=_f32)
                    for jj in range(qn):
                        (t, soff, k_) = chunks[q + jj]
                        nc.tensor.matmul(
                            out=pv[:k_, jj * H : (jj + 1) * H],
                            lhsT=feats_all[:, soff : soff + k_],
                            rhs=w96_t[:],
                            start=True,
                            stop=True,
                        )
                    nc.scalar.copy(
                        out=v_t[:, q * H : (q + qn) * H], in_=pv[:, : qn * H]
                    )

            # one-hots are built lazily (tensor_scalar: int16 iota vs f32
            # per-partition scalar -> bf16, hits the DVE 4x mode), in the
            # exact order the scatter matmuls consume them; some go to the
            # otherwise-idle GpSimd engine
            oh_n = [0]

            def oh_rhs(t, lo, hi, k_):
                oh = oh_p.tile([P, hi - lo], dtype=_f16, tag="oh", name="oh")
                eng = nc.gpsimd if oh_n[0] % 4 == 3 else nc.vector
                oh_n[0] += 1
                eng.tensor_scalar(
                    out=oh[:k_],
                    in0=iota_t[:k_, lo:hi],
                    scalar1=rc_t[:k_, t : t + 1],
                    scalar2=None,
                    op0=mybir.AluOpType.is_equal,
                )
                return oh[:k_]

            live_by_w = dict(wplans)
            # process windows in adjacent groups sharing one multi-bank PSUM
            # tile, so each slab copy covers the whole group
            for wi in range(0, len(live_w), 2):
                wgroup = live_w[wi : wi + 2]
                if wgroup != list(range(wgroup[0], wgroup[0] + len(wgroup))):
                    wgroup = wgroup[:1]  # non-adjacent: fall back to single
                ng = len(wgroup)
                ps = pw_p.tile([P, ng * WIN], dtype=_f32, tag="ps", name="ps")
                for wj, w in enumerate(wgroup):
                    off = wj * WIN
                    went = live_by_w.get(w, [])
                    # zero PSUM column ranges no matmul will write
                    # (vector engine only — GpSimd can't touch PSUM)
                    for gg in range(2):
                        covered = sorted(
                            (lo, hi) for (g2, lo, hi, _) in went if g2 == gg
                        )
                        pos_ = 0
                        for (lo, hi) in covered + [(WIN, WIN)]:
                            if lo > pos_:
                                nc.vector.memset(
                                    ps[gg * H : (gg + 1) * H, off + pos_ : off + lo],
                                    0.0,
                                )
                            pos_ = max(pos_, hi)
                    for (gg, lo, hi, cl) in went:
                        for cc, (t, _, k_) in enumerate(cl):
                            lt = t - t0_pair
                            nc.tensor.matmul(
                                out=ps[gg * H : (gg + 1) * H, off + lo : off + hi],
                                lhsT=v_t[:k_, lt * H : (lt + 1) * H],
                                rhs=oh_rhs(t, lo, hi, k_),
                                start=(cc == 0),
                                stop=(cc == len(cl) - 1),
                            )
                w0 = wgroup[0]
                if (wi // 2) % 2 == 1:
                    nc.vector.tensor_copy(
                        out=slab[:, w0 * WIN : (w0 + ng) * WIN], in_=ps[:]
                    )
                else:
                    nc.scalar.copy(
                        out=slab[:, w0 * WIN : (w0 + ng) * WIN], in_=ps[:]
                    )

            rows = slice(pair * P, (pair + 1) * P)
            for w0 in range(0, w_hi, wgrp):
                c0 = w0 * WIN
                c1 = min((w0 + wgrp) * WIN, live_cells)
                nc.sync.dma_start(out=out_v[rows, c0:c1], in_=slab[:, c0:c1])

    nc.compile()
    return nc


def _prepare(inputs):
    """Host prep + (cached) program build.  Returns (nc, in_maps, mask)."""
    feats_cores, rc_cores, W96, plan, T, live_w, cw, mask = _host_prep(inputs)

    plan_key = (
        T,
        cw,
        tuple(
            (
                t0,
                s0,
                tuple(
                    (w, tuple((gg, lo, hi, tuple(cl)) for (gg, lo, hi, cl) in went))
                    for (w, went) in wplans
                ),
            )
            for (t0, s0, wplans) in plan
        ),
        tuple(live_w),
    )
    nc = _program_cache.get(plan_key)
    if nc is None:
        nc = _build_program(plan, T, live_w, cw)
        _program_cache[plan_key] = nc

    in_maps = [
        {"feats": feats_cores[k], "rc": rc_cores[k], "w96": W96}
        for k in range(NCORES)
    ]
    return nc, in_maps, mask


def kernel(**inputs):
    nc, in_maps, mask = _prepare(inputs)
    res = run_bass_kernel_spmd(nc, in_maps, core_ids=list(range(NCORES)))
    global _last_results
    _last_results = res
    live = np.concatenate([r["out"] for r in res.results], axis=0)
    _, _, rl, cwc = live.shape
    dense = np.zeros((B, H, N, N), np.float32)
    dense[:, :, :rl, :cwc] = live.astype(np.float32)
    return dense, mask


_last_results = None
